# revision 11
# baseline (speedup 1.0000x reference)
"""2-layer dense GCN on 8 Trainium2 NeuronCores — fp8-A streaming version.

Reference computation (all fp32):
    H0 = relu((A_norm @ X) @ W0)
    H1 = relu((A_norm @ H0) @ W1)
A_norm: [16384, 16384], X: [16384, 128], W0/W1: [128, 128].

Sharding: 1D row partition of A_norm (2048 rows/core). Each core streams
its A^T shard quantized to fp8e4 (scaled by 2^13; W pre-divided by the
scale on host), with the stationary X/H operand held in bf16 SBUF tiles.
The tensor engine runs mixed-dtype matmuls (bf16 lhsT x fp8 rhs) at bf16
speed, so halving the A bytes moves the kernel from DMA-bound to the
compute/DMA ridge (PE ~131us/layer at the P0 2.0 GHz clock, A stream
~115us/layer).

Schedule:
- Layer 0 runs CHUNK-MAJOR (4 x 512-wide output chunks, each completing
  its full 16384-deep contraction in rank-major j-order so the
  stationary X loads trickle in behind the A stream). Each finished
  chunk's linear+relu output is AllGathered immediately; the collective
  chain (which cannot start before the ~75us launch-skew barrier)
  overlaps layer-0/1 compute.
- Layer 1 runs PIECE-OUTER: for each AllGather piece p, it feeds piece
  p's j-tiles into all 4 output chunks' PSUM accumulators (4 banks open
  simultaneously). Demand for piece p is at L0_end + p*33us while the
  serialized AG chain supplies it at ~87 + p*25us, so the collectives
  are fully hidden.
- AG triggers are issued on gpsimd BEFORE the previous piece's
  stationary-H loads so triggers never queue behind DMA issues that
  block on collective completion.

Numerics (host-simulated end-to-end): rel err ~4.3e-3 vs fp32 reference
(A fp8e4 + bf16 stationaries/aggregates/weights); measured 5.0e-3 on HW.
"""

import sys
from contextlib import ExitStack

if "/opt/trn_rl_repo" not in sys.path:
    sys.path.insert(0, "/opt/trn_rl_repo")

import numpy as np

N_NODES = 16384
D = 128
NCORES = 8
ROWS = N_NODES // NCORES     # 2048 output rows per core
JT = N_NODES // 128          # 128 contraction j-tiles
JPR = JT // NCORES           # 16 j-tiles per source rank
NCH = 4                      # output chunks / AG pieces per core
IC = ROWS // NCH             # 512 chunk width (one fp32 PSUM bank)
PJT = IC // 128              # 4 j-tiles per rank per piece
A_BUFS = 12                  # A-stream SBUF ring (12 MiB prefetch depth)
SCALE = 2.0 ** 13            # A quantization scale; W is divided by it

# layer-0 chunk-0 DMA group sizes (j-tiles): small first transfers so the
# first matmul can start early; steady state 16-tile (1 MiB) groups
# (larger transfers amortize per-descriptor overhead: 512 KB ran at
# ~244 GB/s effective vs ~318 GB/s for the 2 MiB baseline).
GJ0 = [2, 2, 4, 8] + [16] * 7
GJS = [16] * 8               # all other layer-0 chunks
GJ1 = [16] * 2               # per (piece, chunk) block in layer 1

PRECISION = "fp8"


def _l0_stream():
    """Layer-0 A stream: per chunk, rank-major j order, grouped."""
    stream = []  # (chunk, [j-tiles], width)
    for c in range(NCH):
        gjs = GJ0 if c == 0 else GJS
        k = 0
        for gj in gjs:
            js = [k + t for t in range(gj)]  # j-tile index == stream pos
            stream.append((c, js, IC))
            k += gj
    return stream


def _l1_stream():
    """Layer-1 A stream: piece-outer, then output chunk, then j-tiles."""
    stream = []
    for p in range(NCH):
        pj = [r * JPR + p * PJT + tl for r in range(NCORES) for tl in range(PJT)]
        for c in range(NCH):
            k = 0
            for gj in GJ1:
                stream.append((p, c, pj[k : k + gj]))
                k += gj
    return stream


def build_gcn():
    import concourse.bass as bass  # noqa: F401
    import concourse.tile as tile
    from concourse import bacc, mybir

    F32 = mybir.dt.float32
    BF16 = mybir.dt.bfloat16
    F8 = mybir.dt.float8e4

    nc = bacc.Bacc("TRN2", target_bir_lowering=False, num_devices=NCORES)

    n0 = sum(len(js) for _, js, _ in _l0_stream())   # 512 j-tile slots
    rows0 = len(_l0_stream()) * 128
    rows1 = len(_l1_stream()) * 128
    a0_in = nc.dram_tensor("a0", [rows0, 16 * IC], F8, kind="ExternalInput")
    a1_in = nc.dram_tensor("a1", [rows1, 16 * IC], F8, kind="ExternalInput")
    x_in = nc.dram_tensor("x0", [NCORES * 128, ROWS], BF16, kind="ExternalInput")
    w0 = nc.dram_tensor("w0", [D, D], BF16, kind="ExternalInput")  # W0 / SCALE
    w1 = nc.dram_tensor("w1", [D, D], BF16, kind="ExternalInput")  # W1 / SCALE
    h_out = nc.dram_tensor("h_out", [ROWS, D], F32, kind="ExternalOutput")
    assert n0 == JT * NCH

    relu = mybir.ActivationFunctionType.Relu

    with tile.TileContext(nc) as tc, ExitStack() as ctx:
        sb1 = ctx.enter_context(tc.tile_pool(name="sb1", bufs=1))
        sx_pool = ctx.enter_context(tc.tile_pool(name="sx", bufs=1))
        sh_pool = ctx.enter_context(tc.tile_pool(name="sh", bufs=1))
        a_pool = ctx.enter_context(tc.tile_pool(name="a", bufs=A_BUFS))
        m_pool = ctx.enter_context(tc.tile_pool(name="m", bufs=2))
        h_pool = ctx.enter_context(tc.tile_pool(name="h", bufs=4))
        agg_pool = ctx.enter_context(tc.tile_pool(name="agg", bufs=2, space="PSUM"))
        ag1_pool = ctx.enter_context(tc.tile_pool(name="agg1", bufs=1, space="PSUM"))
        lin_pool = ctx.enter_context(tc.tile_pool(name="lin", bufs=2, space="PSUM"))
        dram = ctx.enter_context(tc.tile_pool(name="dram", bufs=1, space="DRAM"))

        dma_ctr = [0]

        def a_eng():
            eng = nc.sync if dma_ctr[0] % 2 == 0 else nc.scalar
            dma_ctr[0] += 1
            return eng

        w0_sb = sb1.tile([D, D], BF16, name="w0_sb", tag="w0")
        nc.scalar.dma_start(out=w0_sb[:], in_=w0[:])
        w1_sb = sb1.tile([D, D], BF16, name="w1_sb", tag="w1")
        nc.scalar.dma_start(out=w1_sb[:], in_=w1[:])

        # stationary X chunks (bf16); rank r first needed ~8us*r into the
        # stream, so load rank-by-rank (first rank in two halves).
        stat_x = [
            sx_pool.tile([128, ROWS], BF16, name=f"sx{r}", tag=f"sx{r}")
            for r in range(NCORES)
        ]
        for r in range(NCORES):
            splits = [0, ROWS // 4, ROWS] if r == 0 else [0, ROWS]
            for c0, c1 in zip(splits[:-1], splits[1:]):
                nc.gpsimd.dma_start(
                    out=stat_x[r][:, c0:c1],
                    in_=x_in[r * 128 : (r + 1) * 128, c0:c1],
                )

        # stationary H0 tiles, one per (rank, piece), filled as AGs land
        sh = {
            (r, p): sh_pool.tile([128, IC], BF16, name=f"sh{r}_{p}", tag=f"sh{r}_{p}")
            for p in range(NCH)
            for r in range(NCORES)
        }

        h_tb = [
            dram.tile([128, IC], BF16, name=f"htb{p}", tag=f"tb{p}")
            for p in range(NCH)
        ]
        h_ag = [
            dram.tile(
                [NCORES * 128, IC], BF16, addr_space="Shared",
                name=f"hag{p}", tag=f"ag{p}",
            )
            for p in range(NCH)
        ]

        def load_stats(p):
            for r in range(NCORES):
                nc.gpsimd.dma_start(
                    out=sh[(r, p)][:], in_=h_ag[p][r * 128 : (r + 1) * 128, :]
                )

        def linear(agg, w_sb, emit_out, cc):
            mt = m_pool.tile([128, IC], BF16, name="mt", tag="mt")
            nc.vector.tensor_copy(out=mt[:], in_=agg[:])
            for it in range(IC // 128):
                lp = lin_pool.tile([128, D], F32, name="lp", tag="lp")
                nc.tensor.matmul(
                    lp[:],
                    lhsT=mt[:, it * 128 : (it + 1) * 128],
                    rhs=w_sb[:],
                    start=True,
                    stop=True,
                )
                emit_out(cc, it, lp)

        # ---- layer 0 ----
        def emit0(cc, it, lp):
            hh = h_pool.tile([128, D], BF16, name="hh", tag="hh")
            nc.scalar.activation(hh[:], lp[:], relu)
            nc.scalar.dma_start(out=h_tb[cc][:, it * 128 : (it + 1) * 128], in_=hh[:])

        row = 0
        cur = {"agg": None, "k": 0}
        for cc, js, w in _l0_stream():
            if cur["k"] == 0:
                cur["agg"] = agg_pool.tile([128, IC], F32, name="ps", tag="ps")
            at = a_pool.tile([128, len(js) * IC], F8, name="at", tag="at")
            a_eng().dma_start(
                out=at[:], in_=a0_in[row : row + 128, : len(js) * IC]
            )
            row += 128
            for t, j in enumerate(js):
                r, jl = j // JPR, j % JPR
                nc.tensor.matmul(
                    cur["agg"][:],
                    lhsT=stat_x[r][:, jl * 128 : (jl + 1) * 128],
                    rhs=at[:, t * IC : (t + 1) * IC],
                    start=(cur["k"] == 0),
                    stop=(cur["k"] == JT - 1),
                )
                cur["k"] += 1
            if cur["k"] == JT:
                linear(cur["agg"], w0_sb, emit0, cc)
                nc.gpsimd.collective_compute(
                    "AllGather",
                    mybir.AluOpType.bypass,
                    replica_groups=[list(range(NCORES))],
                    ins=[h_tb[cc][:]],
                    outs=[h_ag[cc][:]],
                )
                if cc > 0:
                    load_stats(cc - 1)  # after AG trigger: issues block on
                    # AG(cc-1) completion, never delaying this trigger
                cur["k"] = 0
        load_stats(NCH - 1)

        # ---- layer 1 (piece-outer: all 4 output chunks accumulate) ----
        agg1 = [
            ag1_pool.tile([128, IC], F32, name=f"ps1_{c}", tag=f"ps1_{c}")
            for c in range(NCH)
        ]

        def emit1(cc, it, lp):
            ht = h_pool.tile([128, D], F32, name="ht", tag="ht")
            nc.scalar.activation(ht[:], lp[:], relu)
            nc.scalar.dma_start(
                out=h_out[cc * IC + it * 128 : cc * IC + (it + 1) * 128, :],
                in_=ht[:],
            )

        row = 0
        kc = [0] * NCH  # per-output-chunk j progress
        for p, c, js in _l1_stream():
            at = a_pool.tile([128, len(js) * IC], F8, name="at", tag="at")
            a_eng().dma_start(out=at[:], in_=a1_in[row : row + 128, :])
            row += 128
            for t, j in enumerate(js):
                r = j // JPR
                tl = j % JPR - p * PJT
                nc.tensor.matmul(
                    agg1[c][:],
                    lhsT=sh[(r, p)][:, tl * 128 : (tl + 1) * 128],
                    rhs=at[:, t * IC : (t + 1) * IC],
                    start=(kc[c] == 0),
                    stop=(kc[c] == JT - 1),
                )
                kc[c] += 1
            if kc[c] == JT:
                linear(agg1[c], w1_sb, emit1, c)

    nc.finalize()
    return nc


def _tile_stat(X):
    return np.ascontiguousarray(
        X.reshape(NCORES, JPR, 128, D).transpose(0, 2, 1, 3)
        .reshape(NCORES * 128, ROWS)
    )


def _tile_a(aq4, stream, width_of):
    """aq4: [JT, 128, 2048] quantized A^T tiles. Lay out DMA-group rows
    [128, gj*w] (padded to 8*IC columns) following the stream order."""
    nrow = len(stream) * 128
    out = np.zeros((nrow, 16 * IC), dtype=aq4.dtype)
    for i, item in enumerate(stream):
        js, off, w = width_of(item)
        blk = aq4[js][:, :, off : off + w]          # [gj, 128, w]
        out[i * 128 : (i + 1) * 128, : len(js) * w] = (
            blk.transpose(1, 0, 2).reshape(128, len(js) * w)
        )
    return out


def shard_inputs(A_norm, X, W0, W1):
    """Host-side shard prep. Returns per-core input maps."""
    import ml_dtypes

    bf16 = ml_dtypes.bfloat16
    e4 = ml_dtypes.float8_e4m3

    x_t = _tile_stat(X).astype(bf16)
    w0b = (W0 / SCALE).astype(bf16)
    w1b = (W1 / SCALE).astype(bf16)
    s0, s1 = _l0_stream(), _l1_stream()

    in_maps = []
    for c in range(NCORES):
        a_tc = A_norm[c * ROWS : (c + 1) * ROWS, :].T  # [16384, 2048] view
        aq4 = (a_tc * np.float32(SCALE)).astype(e4).reshape(JT, 128, ROWS)
        a0 = _tile_a(aq4, s0, lambda it: (it[1], it[0] * IC, IC))
        a1 = _tile_a(aq4, s1, lambda it: (it[2], it[1] * IC, IC))
        in_maps.append({"a0": a0, "a1": a1, "x0": x_t, "w0": w0b, "w1": w1b})
    return in_maps


_CACHED = {}


def kernel(A_norm, X, W0, W1):
    A_norm = np.ascontiguousarray(A_norm, dtype=np.float32)
    X = np.ascontiguousarray(X, dtype=np.float32)
    W0 = np.ascontiguousarray(W0, dtype=np.float32)
    W1 = np.ascontiguousarray(W1, dtype=np.float32)

    from concourse.bass_utils import run_bass_kernel_spmd

    if PRECISION not in _CACHED:
        _CACHED[PRECISION] = build_gcn()
    nc = _CACHED[PRECISION]

    in_maps = shard_inputs(A_norm, X, W0, W1)
    res = run_bass_kernel_spmd(nc, in_maps, core_ids=list(range(NCORES)))
    return np.concatenate([res.results[c]["h_out"] for c in range(NCORES)], axis=0)


# revision 15
# speedup vs baseline: 1.0244x; 1.0244x over previous
"""2-layer dense GCN on 8 Trainium2 NeuronCores — fp8-A streaming version.

Reference computation (all fp32):
    H0 = relu((A_norm @ X) @ W0)
    H1 = relu((A_norm @ H0) @ W1)
A_norm: [16384, 16384], X: [16384, 128], W0/W1: [128, 128].

Sharding: 1D row partition of A_norm (2048 rows/core). Each core streams
its A^T shard quantized to fp8e4 (scaled by 2^13; W pre-divided by the
scale on host), with the stationary X/H operand held in bf16 SBUF tiles.
The tensor engine runs mixed-dtype matmuls (bf16 lhsT x fp8 rhs) at bf16
speed, so halving the A bytes moves the kernel from DMA-bound to the
compute/DMA ridge (PE ~131us/layer at the P0 2.0 GHz clock, A stream
~115us/layer).

Schedule:
- Layer 0 runs CHUNK-MAJOR (4 x 512-wide output chunks, each completing
  its full 16384-deep contraction in rank-major j-order so the
  stationary X loads trickle in behind the A stream). Each finished
  chunk's linear+relu output is AllGathered immediately; the collective
  chain (which cannot start before the ~75us launch-skew barrier)
  overlaps layer-0/1 compute.
- Layer 1 runs PIECE-OUTER: for each AllGather piece p, it feeds piece
  p's j-tiles into all 4 output chunks' PSUM accumulators (4 banks open
  simultaneously). Demand for piece p is at L0_end + p*33us while the
  serialized AG chain supplies it at ~87 + p*25us, so the collectives
  are fully hidden.
- AG triggers are issued on gpsimd BEFORE the previous piece's
  stationary-H loads so triggers never queue behind DMA issues that
  block on collective completion.

Numerics (host-simulated end-to-end): rel err ~4.3e-3 vs fp32 reference
(A fp8e4 + bf16 stationaries/aggregates/weights); measured 5.0e-3 on HW.
"""

import sys
from contextlib import ExitStack

if "/opt/trn_rl_repo" not in sys.path:
    sys.path.insert(0, "/opt/trn_rl_repo")

import numpy as np

N_NODES = 16384
D = 128
NCORES = 8
ROWS = N_NODES // NCORES     # 2048 output rows per core
JT = N_NODES // 128          # 128 contraction j-tiles
JPR = JT // NCORES           # 16 j-tiles per source rank
NCH = 4                      # output chunks / AG pieces per core
IC = ROWS // NCH             # 512 chunk width (one fp32 PSUM bank)
PJT = IC // 128              # 4 j-tiles per rank per piece
A_BUFS = 16                  # A-stream SBUF ring (8 MiB prefetch depth)
SCALE = 2.0 ** 13            # A quantization scale; W is divided by it
HCARRY = 256.0               # H0 carried as HCARRY*H0 in fp8e4 transport

# layer-0 chunk-0 DMA group sizes (j-tiles): small first transfers so the
# first matmul can start early; steady state 8-tile (512 KB) groups
# (1 MiB groups measured slower end-to-end: burstier PE waits).
GJ0 = [2, 2, 4] + [8] * 15
GJS = [8] * 16               # all other layer-0 chunks
GJ1 = [8] * 4                # per (piece, chunk) block in layer 1

PRECISION = "fp8"


def _l0_stream():
    """Layer-0 A stream: per chunk, rank-major j order, grouped."""
    stream = []  # (chunk, [j-tiles], width)
    for c in range(NCH):
        gjs = GJ0 if c == 0 else GJS
        k = 0
        for gj in gjs:
            js = [k + t for t in range(gj)]  # j-tile index == stream pos
            stream.append((c, js, IC))
            k += gj
    return stream


def _l1_stream():
    """Layer-1 A stream: piece-outer, then output chunk, then j-tiles."""
    stream = []
    for p in range(NCH):
        pj = [r * JPR + p * PJT + tl for r in range(NCORES) for tl in range(PJT)]
        for c in range(NCH):
            k = 0
            for gj in GJ1:
                stream.append((p, c, pj[k : k + gj]))
                k += gj
    return stream


def build_gcn():
    import concourse.bass as bass  # noqa: F401
    import concourse.tile as tile
    from concourse import bacc, mybir

    F32 = mybir.dt.float32
    BF16 = mybir.dt.bfloat16
    F8 = mybir.dt.float8e4

    nc = bacc.Bacc("TRN2", target_bir_lowering=False, num_devices=NCORES)

    n0 = sum(len(js) for _, js, _ in _l0_stream())   # 512 j-tile slots
    rows0 = len(_l0_stream()) * 128
    rows1 = len(_l1_stream()) * 128
    gmax = max(GJ0 + GJS + GJ1)
    a0_in = nc.dram_tensor("a0", [rows0, gmax * IC], F8, kind="ExternalInput")
    a1_in = nc.dram_tensor("a1", [rows1, gmax * IC], F8, kind="ExternalInput")
    x_in = nc.dram_tensor("x0", [NCORES * 128, ROWS], BF16, kind="ExternalInput")
    w0 = nc.dram_tensor("w0", [D, D], BF16, kind="ExternalInput")  # W0 / SCALE
    w1 = nc.dram_tensor("w1", [D, D], BF16, kind="ExternalInput")  # W1 / SCALE
    h_out = nc.dram_tensor("h_out", [ROWS, D], F32, kind="ExternalOutput")
    assert n0 == JT * NCH

    relu = mybir.ActivationFunctionType.Relu

    with tile.TileContext(nc) as tc, ExitStack() as ctx:
        sb1 = ctx.enter_context(tc.tile_pool(name="sb1", bufs=1))
        sx_pool = ctx.enter_context(tc.tile_pool(name="sx", bufs=1))
        sh_pool = ctx.enter_context(tc.tile_pool(name="sh", bufs=1))
        a_pool = ctx.enter_context(tc.tile_pool(name="a", bufs=A_BUFS))
        m_pool = ctx.enter_context(tc.tile_pool(name="m", bufs=2))
        h_pool = ctx.enter_context(tc.tile_pool(name="h", bufs=4))
        agg_pool = ctx.enter_context(tc.tile_pool(name="agg", bufs=2, space="PSUM"))
        ag1_pool = ctx.enter_context(tc.tile_pool(name="agg1", bufs=1, space="PSUM"))
        lin_pool = ctx.enter_context(tc.tile_pool(name="lin", bufs=2, space="PSUM"))
        dram = ctx.enter_context(tc.tile_pool(name="dram", bufs=1, space="DRAM"))

        dma_ctr = [0]

        def a_eng():
            eng = nc.sync if dma_ctr[0] % 2 == 0 else nc.scalar
            dma_ctr[0] += 1
            return eng

        w0_sb = sb1.tile([D, D], BF16, name="w0_sb", tag="w0")
        nc.scalar.dma_start(out=w0_sb[:], in_=w0[:])
        w1_sb = sb1.tile([D, D], BF16, name="w1_sb", tag="w1")
        nc.scalar.dma_start(out=w1_sb[:], in_=w1[:])

        # stationary X chunks (bf16); rank r first needed ~8us*r into the
        # stream, so load rank-by-rank (first rank in two halves).
        stat_x = [
            sx_pool.tile([128, ROWS], BF16, name=f"sx{r}", tag=f"sx{r}")
            for r in range(NCORES)
        ]
        for r in range(NCORES):
            splits = [0, ROWS // 4, ROWS] if r == 0 else [0, ROWS]
            for c0, c1 in zip(splits[:-1], splits[1:]):
                nc.gpsimd.dma_start(
                    out=stat_x[r][:, c0:c1],
                    in_=x_in[r * 128 : (r + 1) * 128, c0:c1],
                )

        # stationary H0 tiles, one per (rank, piece), filled as AGs land
        sh = {
            (r, p): sh_pool.tile([128, IC], F8, name=f"sh{r}_{p}", tag=f"sh{r}_{p}")
            for p in range(NCH)
            for r in range(NCORES)
        }

        h_tb = [
            dram.tile([128, IC], F8, name=f"htb{p}", tag=f"tb{p}")
            for p in range(NCH)
        ]
        h_ag = [
            dram.tile(
                [NCORES * 128, IC], F8, addr_space="Shared",
                name=f"hag{p}", tag=f"ag{p}",
            )
            for p in range(NCH)
        ]

        def load_stats(p):
            for r in range(NCORES):
                nc.gpsimd.dma_start(
                    out=sh[(r, p)][:], in_=h_ag[p][r * 128 : (r + 1) * 128, :]
                )

        def linear(agg, w_sb, emit_out, cc):
            mt = m_pool.tile([128, IC], BF16, name="mt", tag="mt")
            nc.vector.tensor_copy(out=mt[:], in_=agg[:])
            for it in range(IC // 128):
                lp = lin_pool.tile([128, D], F32, name="lp", tag="lp")
                nc.tensor.matmul(
                    lp[:],
                    lhsT=mt[:, it * 128 : (it + 1) * 128],
                    rhs=w_sb[:],
                    start=True,
                    stop=True,
                )
                emit_out(cc, it, lp)

        # ---- layer 0 ----
        def emit0(cc, it, lp):
            hh = h_pool.tile([128, D], F8, name="hh", tag="hh")
            nc.scalar.activation(hh[:], lp[:], relu)
            nc.scalar.dma_start(out=h_tb[cc][:, it * 128 : (it + 1) * 128], in_=hh[:])

        row = 0
        cur = {"agg": None, "k": 0}
        for cc, js, w in _l0_stream():
            if cur["k"] == 0:
                cur["agg"] = agg_pool.tile([128, IC], F32, name="ps", tag="ps")
            at = a_pool.tile([128, len(js) * IC], F8, name="at", tag="at")
            a_eng().dma_start(
                out=at[:], in_=a0_in[row : row + 128, : len(js) * IC]
            )
            row += 128
            for t, j in enumerate(js):
                r, jl = j // JPR, j % JPR
                nc.tensor.matmul(
                    cur["agg"][:],
                    lhsT=stat_x[r][:, jl * 128 : (jl + 1) * 128],
                    rhs=at[:, t * IC : (t + 1) * IC],
                    start=(cur["k"] == 0),
                    stop=(cur["k"] == JT - 1),
                )
                cur["k"] += 1
            if cur["k"] == JT:
                linear(cur["agg"], w0_sb, emit0, cc)
                nc.gpsimd.collective_compute(
                    "AllGather",
                    mybir.AluOpType.bypass,
                    replica_groups=[list(range(NCORES))],
                    ins=[h_tb[cc][:]],
                    outs=[h_ag[cc][:]],
                )
                if cc > 0:
                    load_stats(cc - 1)  # after AG trigger: issues block on
                    # AG(cc-1) completion, never delaying this trigger
                cur["k"] = 0
        load_stats(NCH - 1)

        # ---- layer 1 (piece-outer: all 4 output chunks accumulate) ----
        agg1 = [
            ag1_pool.tile([128, IC], F32, name=f"ps1_{c}", tag=f"ps1_{c}")
            for c in range(NCH)
        ]

        def emit1(cc, it, lp):
            ht = h_pool.tile([128, D], F32, name="ht", tag="ht")
            nc.scalar.activation(ht[:], lp[:], relu)
            nc.scalar.dma_start(
                out=h_out[cc * IC + it * 128 : cc * IC + (it + 1) * 128, :],
                in_=ht[:],
            )

        row = 0
        kc = [0] * NCH  # per-output-chunk j progress
        for p, c, js in _l1_stream():
            at = a_pool.tile([128, len(js) * IC], F8, name="at", tag="at")
            a_eng().dma_start(
                out=at[:], in_=a1_in[row : row + 128, : len(js) * IC]
            )
            row += 128
            for t, j in enumerate(js):
                r = j // JPR
                tl = j % JPR - p * PJT
                nc.tensor.matmul(
                    agg1[c][:],
                    lhsT=sh[(r, p)][:, tl * 128 : (tl + 1) * 128],
                    rhs=at[:, t * IC : (t + 1) * IC],
                    start=(kc[c] == 0),
                    stop=(kc[c] == JT - 1),
                )
                kc[c] += 1
            if kc[c] == JT:
                linear(agg1[c], w1_sb, emit1, c)

    nc.finalize()
    return nc


def _tile_stat(X):
    return np.ascontiguousarray(
        X.reshape(NCORES, JPR, 128, D).transpose(0, 2, 1, 3)
        .reshape(NCORES * 128, ROWS)
    )


def _tile_a(aq4, stream, width_of):
    """aq4: [JT, 128, 2048] quantized A^T tiles. Lay out DMA-group rows
    [128, gj*w] (padded to 8*IC columns) following the stream order."""
    nrow = len(stream) * 128
    gmax = max(GJ0 + GJS + GJ1)
    out = np.zeros((nrow, gmax * IC), dtype=aq4.dtype)
    for i, item in enumerate(stream):
        js, off, w = width_of(item)
        blk = aq4[js][:, :, off : off + w]          # [gj, 128, w]
        out[i * 128 : (i + 1) * 128, : len(js) * w] = (
            blk.transpose(1, 0, 2).reshape(128, len(js) * w)
        )
    return out


def shard_inputs(A_norm, X, W0, W1):
    """Host-side shard prep. Returns per-core input maps."""
    import ml_dtypes

    bf16 = ml_dtypes.bfloat16
    e4 = ml_dtypes.float8_e4m3

    x_t = _tile_stat(X).astype(bf16)
    w0b = (W0 * (HCARRY / SCALE)).astype(bf16)
    w1b = (W1 / (SCALE * HCARRY)).astype(bf16)
    s0, s1 = _l0_stream(), _l1_stream()

    in_maps = []
    for c in range(NCORES):
        a_tc = A_norm[c * ROWS : (c + 1) * ROWS, :].T  # [16384, 2048] view
        aq4 = (a_tc * np.float32(SCALE)).astype(e4).reshape(JT, 128, ROWS)
        a0 = _tile_a(aq4, s0, lambda it: (it[1], it[0] * IC, IC))
        a1 = _tile_a(aq4, s1, lambda it: (it[2], it[1] * IC, IC))
        in_maps.append({"a0": a0, "a1": a1, "x0": x_t, "w0": w0b, "w1": w1b})
    return in_maps


_CACHED = {}


def kernel(A_norm, X, W0, W1):
    A_norm = np.ascontiguousarray(A_norm, dtype=np.float32)
    X = np.ascontiguousarray(X, dtype=np.float32)
    W0 = np.ascontiguousarray(W0, dtype=np.float32)
    W1 = np.ascontiguousarray(W1, dtype=np.float32)

    from concourse.bass_utils import run_bass_kernel_spmd

    if PRECISION not in _CACHED:
        _CACHED[PRECISION] = build_gcn()
    nc = _CACHED[PRECISION]

    in_maps = shard_inputs(A_norm, X, W0, W1)
    res = run_bass_kernel_spmd(nc, in_maps, core_ids=list(range(NCORES)))
    return np.concatenate([res.results[c]["h_out"] for c in range(NCORES)], axis=0)
